# revision 57
# baseline (speedup 1.0000x reference)
"""Self-contained Trainium2 Bass kernel for nn_MultiHeadAttention_68367289417808.

kernel(**inputs) takes FULL unsharded inputs (as in reference.setup_inputs())
and returns the FULL [4, 2048, 1024] output.

Sharding: 8 cores = (batch 4) x (query-half 2); no collectives needed.

Per-core pipeline (all matmuls bf16, PSUM fp32):
  - q/k/v/mask/weights SBUF-resident (no DRAM scratch roundtrips)
  - the reference's masked_fill(-1e-6) semantics give unnormalized weights
    u = m*exp(s) + (1-m).  We compute w = (exp(s)-1)*m on the vector engine
    (one scalar_tensor_tensor per tile, mask stored as fp8 0/1) so u = w + 1,
    and fold the "+1" into a host-precomputed rank-1 correction:
    numerator += colsum(v@Wv), denominator += SK.  No mask matmul and no
    mask@V correction on PE; bv's effect folds into the output bias on host.
  - Q projection runs upfront under the input DMA shadow; per-head-pair K/V
    projections interleave into the previous pair's attention t-loop as PE
    filler; attn@V trails scores by ~4 steps in one continuous software
    pipeline across all (pair, head, t) steps; exp on Act, softmax
    normalization spread across DVE/Act/GPSIMD.
  - output projection + bias; per-core [1024, 1024] slices assembled on host.
"""
import time
from collections import deque

import jax
import numpy as np
from jax.experimental.shard_map import shard_map
from jax.sharding import Mesh, PartitionSpec

import concourse.bass as bass
import concourse.bacc as bacc
import concourse.mybir as mybir
import concourse.tile as tile
from concourse import bass2jax
from concourse.bass import ts, ds

F32 = mybir.dt.float32
BF16 = mybir.dt.bfloat16
FP8 = mybir.dt.float8e4
BF16_NP = mybir.dt.np(mybir.dt.bfloat16)
FP8_NP = mybir.dt.np(mybir.dt.float8e4)
AF = mybir.ActivationFunctionType
MULT = mybir.AluOpType.mult
ADD = mybir.AluOpType.add

P = 128
SQ = 1024
SK = 2048
D = 1024
H = 16
DK = 64
HK = 1024


def build_mha():
    nc = bacc.Bacc("TRN2", target_bir_lowering=False)

    qT = nc.dram_tensor("qT", [D, SQ], BF16, kind="ExternalInput")
    kT = nc.dram_tensor("kT", [D, SK], BF16, kind="ExternalInput")
    vT = nc.dram_tensor("vT", [D, SK], BF16, kind="ExternalInput")
    mT = nc.dram_tensor("mT", [SK, SQ], FP8, kind="ExternalInput")
    wq = nc.dram_tensor("wq", [D, HK], BF16, kind="ExternalInput")
    wk = nc.dram_tensor("wk", [D, HK], BF16, kind="ExternalInput")
    wv = nc.dram_tensor("wv", [D, HK], BF16, kind="ExternalInput")
    wo = nc.dram_tensor("wo", [HK, D], BF16, kind="ExternalInput")
    bq2 = nc.dram_tensor("bq2", [P, 8], F32, kind="ExternalInput")
    bk2 = nc.dram_tensor("bk2", [P, 8], F32, kind="ExternalInput")
    bor = nc.dram_tensor("bor", [1, D], F32, kind="ExternalInput")
    csum_d = nc.dram_tensor("csum", [65, H], F32, kind="ExternalInput")
    out = nc.dram_tensor("out", [SQ, D], F32, kind="ExternalOutput")

    rden_d = nc.dram_tensor("rden_scr", [H, SQ], F32)

    with tile.TileContext(nc) as tc:
        with tc.tile_pool(name="consts", bufs=1) as cst:
            bq_sb = cst.tile([P, 8], F32, tag="bq")
            bk_sb = cst.tile([P, 8], F32, tag="bk")
            bo_bc = cst.tile([P, D], F32, tag="bo")
            csum_sb = cst.tile([65, H], F32, tag="cs")
            ones2 = cst.tile([P, H, 2], BF16, tag="on")

            nc.sync.dma_start(bq_sb[:], bq2.ap())
            nc.sync.dma_start(bk_sb[:], bk2.ap())
            nc.sync.dma_start(bo_bc[:], bor.ap().to_broadcast((P, D)))
            nc.sync.dma_start(csum_sb[:], csum_d.ap())
            nc.gpsimd.memset(ones2[:], 1.0)

            with tc.tile_pool(name="aop", bufs=1) as aop:
              ao_sb = aop.tile([P, 8, SQ], BF16, tag="ao")
              wo_c0 = aop.tile([P, 8, 512], BF16, tag="woc0")
              with tc.tile_pool(name="res", bufs=1) as res:
                vT_sb = res.tile([P, 8, SK], BF16, tag="vt")
                wv_sb = res.tile([P, 8, HK], BF16, tag="wv")
                kT_sb = res.tile([P, 8, SK], BF16, tag="kt")
                wk_sb = res.tile([P, 8, HK], BF16, tag="wk")
                m_sb = res.tile([P, H, SQ], FP8, tag="m")

                with tc.tile_pool(name="qh8", bufs=8) as qh8p:
                  qh8 = [qh8p.tile([P, SQ], BF16, tag="qh8", name=f"qh{_g}")
                         for _g in range(8)]
                  # Q projection upfront: needs only qT+wq (small DMA), runs
                  # while the big kT/vT/m loads stream in; qT/wq then free.
                  with (
                      tc.tile_pool(name="qtmp", bufs=1) as qtmp,
                      tc.tile_pool(name="qpj", bufs=2, space="PSUM") as qpj,
                  ):
                    qT_sb = qtmp.tile([P, 8, SQ], BF16, tag="qt")
                    wq_sb = qtmp.tile([P, 8, HK], BF16, tag="wq")
                    # DMA issue order matches PE consumption order:
                    # qT(c0)+wq(first pairs) -> Q chunks c0; qT(c1) -> c1;
                    # kT/wk -> pair-0 K chunks; vT/wv -> V; mask last.
                    for _j in range(8):
                        nc.sync.dma_start(
                            qT_sb[:, _j, ts(0, 512)],
                            qT.ap().rearrange("(j p) s -> p j s", p=P)[
                                :, _j, ts(0, 512)
                            ],
                        )
                        nc.sync.dma_start(
                            wq_sb[:, _j],
                            wq.ap().rearrange("(j p) m -> p j m", p=P)[:, _j],
                        )
                    for _j in range(8):
                        nc.sync.dma_start(
                            qT_sb[:, _j, ts(1, 512)],
                            qT.ap().rearrange("(j p) s -> p j s", p=P)[
                                :, _j, ts(1, 512)
                            ],
                        )
                    for _j in range(8):
                        nc.sync.dma_start(
                            wk_sb[:, _j],
                            wk.ap().rearrange("(j p) m -> p j m", p=P)[:, _j],
                        )
                    for half in range(2):
                        for _j in range(8):
                            nc.sync.dma_start(
                                kT_sb[:, _j, ts(half, SK // 2)],
                                kT.ap().rearrange("(j p) s -> p j s", p=P)[
                                    :, _j, ts(half, SK // 2)
                                ],
                            )
                    nc.sync.dma_start(
                        m_sb[:, ds(0, 4), :],
                        mT.ap().rearrange("(t p) s -> p t s", p=P)[:, ds(0, 4), :],
                    )
                    for _j in range(8):
                        nc.sync.dma_start(
                            wv_sb[:, _j],
                            wv.ap().rearrange("(j p) m -> p j m", p=P)[:, _j],
                        )
                    for half in range(2):
                        for _j in range(8):
                            nc.sync.dma_start(
                                vT_sb[:, _j, ts(half, SK // 2)],
                                vT.ap().rearrange("(j p) s -> p j s", p=P)[
                                    :, _j, ts(half, SK // 2)
                                ],
                            )
                        if half == 0:
                            nc.sync.dma_start(
                                m_sb[:, ds(4, 4), :],
                                mT.ap().rearrange("(t p) s -> p t s", p=P)[
                                    :, ds(4, 4), :
                                ],
                            )
                    for c in range(2, 4):
                        nc.sync.dma_start(
                            m_sb[:, ds(4 * c, 4), :],
                            mT.ap().rearrange("(t p) s -> p t s", p=P)[
                                :, ds(4 * c, 4), :
                            ],
                        )
                    for cc in range(2):
                        for gg in range(8):
                            psq = qpj.tile([P, 512], F32, tag="qj")
                            for j in range(8):
                                nc.tensor.matmul(
                                    psq[:],
                                    wq_sb[:, j, ds(P * gg, P)],
                                    qT_sb[:, j, ts(cc, 512)],
                                    start=(j == 0),
                                    stop=(j == 7),
                                )
                            nc.scalar.activation(
                                qh8[gg][:, ts(cc, 512)],
                                psq[:],
                                AF.Identity,
                                bias=bq_sb[:, gg : gg + 1],
                            )

                  with (
                    tc.tile_pool(name="khp", bufs=2) as khp,
                    tc.tile_pool(name="vap", bufs=2) as vap,
                    tc.tile_pool(name="ep", bufs=4) as ep,
                    tc.tile_pool(name="wpb", bufs=7) as wpb,
                    tc.tile_pool(name="osb", bufs=1) as osb,
                    tc.tile_pool(name="rbp", bufs=1) as rbp,
                    tc.tile_pool(name="tnp", bufs=1) as tnp,
                    tc.tile_pool(name="pss", bufs=2, space="PSUM") as pss,
                    tc.tile_pool(name="pso", bufs=1, space="PSUM") as pso,
                    tc.tile_pool(name="ppj", bufs=2, space="PSUM") as ppj,
                ):
                    pair_tiles = {}

                    def make_thunks(gg):
                        """Projection work for head pair gg as a list of
                        thunks, each ~8 matmuls + one vector op."""
                        kh = khp.tile([P, SK], BF16, tag="kh")
                        va = vap.tile([P, H, 130], BF16, tag="va")
                        pair_tiles[gg] = (kh, va)
                        thunks = []

                        def k_chunk(cc):
                            def run():
                                psk = ppj.tile([P, 512], F32, tag="pj")
                                for j in range(8):
                                    nc.tensor.matmul(
                                        psk[:],
                                        wk_sb[:, j, ds(P * gg, P)],
                                        kT_sb[:, j, ts(cc, 512)],
                                        start=(j == 0),
                                        stop=(j == 7),
                                    )
                                nc.scalar.activation(
                                    kh[:, ts(cc, 512)],
                                    psk[:],
                                    AF.Identity,
                                    bias=bk_sb[:, gg : gg + 1],
                                )
                            return run

                        def v_chunk(cc):
                            def run():
                                psv = ppj.tile([P, 512], F32, tag="pj")
                                for tt in range(4):
                                    t = 4 * cc + tt
                                    for j in range(8):
                                        nc.tensor.matmul(
                                            psv[:, ts(tt, P)],
                                            vT_sb[:, j, ts(t, P)],
                                            wv_sb[:, j, ds(P * gg, P)],
                                            start=(j == 0),
                                            stop=(j == 7),
                                        )
                                nc.vector.tensor_copy(
                                    va[:, ds(4 * cc, 4), :].rearrange(
                                        "p t (h k) -> p t h k", h=2
                                    )[:, :, :, 0:64],
                                    psv[:].rearrange(
                                        "p (t h k) -> p t h k", t=4, h=2
                                    ),
                                )
                            return run

                        def v_ones():
                            nc.vector.tensor_copy(va[:, :, 64:130:65], ones2[:])

                        for cc in range(4):
                            thunks.append(k_chunk(cc))
                        for cc in range(4):
                            thunks.append(v_chunk(cc))
                        thunks.append(v_ones)
                        return thunks

                    pending = deque(make_thunks(0))
                    while pending:
                        pending.popleft()()

                    cur = {}
                    pso_tiles = {}

                    def attnv(g_, hh_, t_, w_):
                        va_ = cur[g_][1]
                        ps_o_ = pso_tiles[(g_, hh_)]
                        for c2 in range(2):
                            nc.tensor.matmul(
                                ps_o_[:, ts(c2, 512)],
                                va_[:, t_, ds(65 * hh_, 65)],
                                w_[:, ts(c2, 512)],
                                start=(t_ == 0),
                                stop=(t_ == 15),
                            )

                    def normalize(g_, hh_):
                        # numerator += colsum(vh); denominator = row 64
                        # (+SK via csum row 64); ao = numerator/denominator
                        h_ = 2 * g_ + hh_
                        ps_o_ = pso_tiles.pop((g_, hh_))
                        o_sb = osb.tile([65, SQ], F32, tag="osb")
                        nc.scalar.activation(
                            o_sb[:],
                            ps_o_[:],
                            AF.Identity,
                            bias=csum_sb[:, h_ : h_ + 1],
                        )
                        nc.vector.reciprocal(o_sb[64:65, :], o_sb[64:65, :])
                        nc.sync.dma_start(
                            rden_d.ap()[h_ : h_ + 1, :], o_sb[64:65, :]
                        )
                        rbc = rbp.tile([64, SQ], F32, tag="rbc")
                        nc.sync.dma_start(
                            rbc[:],
                            rden_d.ap()[h_ : h_ + 1, :].to_broadcast((64, SQ)),
                        )
                        if hh_ == 0:
                            nc.gpsimd.tensor_tensor(
                                ao_sb[0:64, g_, :], o_sb[0:64, :], rbc[:], MULT
                            )
                        else:
                            tmpn = tnp.tile([64, SQ], BF16, tag="tn")
                            nc.gpsimd.tensor_tensor(
                                tmpn[:], o_sb[0:64, :], rbc[:], MULT
                            )
                            nc.sync.dma_start(ao_sb[64:128, g_, :], tmpn[:])

                    def drain_one(hist):
                        g_, hh_, t_, w_ = hist.pop(0)
                        attnv(g_, hh_, t_, w_)
                        if t_ == 15:
                            normalize(g_, hh_)

                    def maybe_drain(hist, g, hh, t):
                        # drain prev-(g,hh) items promptly, but defer the
                        # current accumulator's first attn@V until the
                        # single pso slot is freed by the previous
                        # normalize (~slot 3)
                        if len(hist) <= 4:
                            return
                        g_, hh_, t_, _ = hist[0]
                        if (g_, hh_) == (g, hh) and t < 5:
                            return
                        drain_one(hist)

                    # one continuous software-pipelined stream over all
                    # (pair, head, t) steps; attn@V trails scores by 3 steps
                    hist = []
                    slot = 0
                    for g in range(8):
                        if g == 7:
                            # prefetch first half of Wo during the last pair
                            for _j in range(8):
                                nc.sync.dma_start(
                                    wo_c0[:, _j],
                                    wo.ap().rearrange("(j p) m -> p j m", p=P)[
                                        :, _j, 0:512
                                    ],
                                )
                        if g < 7:
                            pending.extend(make_thunks(g + 1))
                        cur[g] = pair_tiles.pop(g)
                        kh, va = cur[g]
                        qh = qh8[g]
                        # hh=1 first: its normalize ends in a partition-shift
                        # DMA; keeping the direct-write head (hh=0) last
                        # shortens the tail into the output projection
                        for hh in (1, 0):
                            base = 64 * hh
                            pso_tiles[(g, hh)] = pso.tile(
                                [65, SQ], F32, tag="pso", name=f"pso_{g}_{hh}"
                            )
                            for t in range(16):
                                ps_s = pss.tile([P, SQ], F32, tag="pss")
                                for c2 in range(2):
                                    nc.tensor.matmul(
                                        ps_s[:, ts(c2, 512)],
                                        kh[base : base + 64, ts(t, P)],
                                        qh[base : base + 64, ts(c2, 512)],
                                        start=True,
                                        stop=True,
                                    )
                                e = ep.tile([P, SQ], BF16, tag="e")
                                nc.scalar.activation(
                                    e[:], ps_s[:], AF.Exp, scale=0.125
                                )
                                # w = (e-1)*m  (masked -> 0; the "+1" is the
                                # rank-1 csum correction applied at normalize)
                                w = wpb.tile([P, SQ], BF16, tag="w")
                                nc.vector.scalar_tensor_tensor(
                                    w[:], e[:], -1.0, m_sb[:, t, :], ADD, MULT
                                )
                                hist.append((g, hh, t, w))
                                maybe_drain(hist, g, hh, t)
                                if slot % 3 == 1 and pending:
                                    pending.popleft()()
                                slot += 1
                        cur.pop(g - 1, None)
                    while hist:
                        drain_one(hist)

              # ---------------- output projection ----------------
              with (
                  tc.tile_pool(name="wop", bufs=1) as wop,
                  tc.tile_pool(name="pp2", bufs=8, space="PSUM") as pp2,
                  tc.tile_pool(name="po2", bufs=3) as po2,
              ):
                  wo_c1 = wop.tile([P, 8, 512], BF16, tag="woc1")
                  for _j in range(8):
                      nc.sync.dma_start(
                          wo_c1[:, _j],
                          wo.ap().rearrange("(j p) m -> p j m", p=P)[
                              :, _j, 512:1024
                          ],
                      )
                  # all c=0 units first (prefetched half), c=1 half streams in
                  scs = [(s, c) for c in range(2) for s in range(8)]
                  for blk in range(4):
                      units = scs[4 * blk : 4 * blk + 4]
                      psos = [
                          pp2.tile([P, 512], F32, tag="op", name=f"pso2_{_u}")
                          for _u in range(len(units))
                      ]
                      for j in range(8):
                          for u, (s, c) in enumerate(units):
                              nc.tensor.matmul(
                                  psos[u][:],
                                  ao_sb[:, j, ts(s, P)],
                                  (wo_c0 if c == 0 else wo_c1)[:, j, :],
                                  start=(j == 0),
                                  stop=(j == 7),
                              )
                      for u, (s, c) in enumerate(units):
                          oo = po2.tile([P, 512], F32, tag="oo")
                          nc.vector.tensor_tensor(
                              oo[:], psos[u][:], bo_bc[:, ts(c, 512)], ADD
                          )
                          nc.sync.dma_start(out.ap()[ts(s, P), ts(c, 512)], oo[:])

    nc.compile()
    return nc


def make_host_inputs(q, k, v, mask, Wq, bq, Wk, bk, Wv, bv, Wo, bo):
    """Full inputs -> list of 8 per-core input dicts."""
    q = np.asarray(q, np.float32)
    k = np.asarray(k, np.float32)
    v = np.asarray(v, np.float32)
    mask = np.asarray(mask)
    Wq = np.asarray(Wq, np.float32)
    Wk = np.asarray(Wk, np.float32)
    Wv = np.asarray(Wv, np.float32)
    Wo = np.asarray(Wo, np.float32)
    bq = np.asarray(bq, np.float32)
    bk = np.asarray(bk, np.float32)
    bv = np.asarray(bv, np.float32)
    bo = np.asarray(bo, np.float32)

    def b16(a):
        return np.ascontiguousarray(a).astype(BF16_NP)

    # bv contributes exactly bv[hk] to each normalized attention output
    # (weights sum to den), so its effect folds into the output bias.
    bo_eff = bo + bv.reshape(HK) @ Wo
    shared = {
        "wq": b16(Wq.transpose(1, 0, 2).reshape(D, HK)),
        "wk": b16(Wk.transpose(1, 0, 2).reshape(D, HK)),
        "wv": b16(Wv.transpose(1, 0, 2).reshape(D, HK)),
        "wo": b16(Wo),
        "bq2": np.ascontiguousarray(bq.reshape(HK).reshape(8, P).T),
        "bk2": np.ascontiguousarray(bk.reshape(HK).reshape(8, P).T),
        "bor": np.ascontiguousarray(bo_eff.reshape(1, D)),
    }

    in_maps = []
    for core in range(8):
        b, j = divmod(core, 2)
        qs = q[b, j * SQ : (j + 1) * SQ, :]
        ms = mask[b, j * SQ : (j + 1) * SQ, :]
        # rank-1 correction: colsum of vh per head; row 64 = +SK on denom
        vsum = v[b].sum(axis=0)  # [D]
        cs = np.einsum("d,hdk->hk", vsum, Wv)  # [H, DK] (vh excludes bv)
        csum = np.empty((65, H), np.float32)
        csum[0:64, :] = cs.T
        csum[64, :] = float(SK)
        m = dict(shared)
        m["qT"] = b16(qs.T)
        m["kT"] = b16(k[b].T)
        m["vT"] = b16(v[b].T)
        m["mT"] = np.ascontiguousarray(ms.T).astype(np.float32).astype(FP8_NP)
        m["csum"] = np.ascontiguousarray(csum)
        in_maps.append(m)
    return in_maps


def assemble_output(results):
    """8 per-core out [SQ, D] -> full [4, 2048, 1024]."""
    B, S = 4, 2048
    full = np.empty((B, S, D), np.float32)
    for core, res in enumerate(results):
        b, j = divmod(core, 2)
        full[b, j * SQ : (j + 1) * SQ, :] = res["out"]
    return full


class CompiledSpmd:
    def __init__(self, nc: bass.Bass, n_cores: int):
        bass2jax.install_neuronx_cc_hook()
        assert nc.dbg_addr is None, "build with debug=False"
        partition_name = (
            nc.partition_id_tensor.name if nc.partition_id_tensor else None
        )
        in_names, out_names, out_avals, zero_outs = [], [], [], []
        for alloc in nc.m.functions[0].allocations:
            if not isinstance(alloc, mybir.MemoryLocationSet):
                continue
            name = alloc.memorylocations[0].name
            if alloc.kind == "ExternalInput":
                if name != partition_name:
                    in_names.append(name)
            elif alloc.kind == "ExternalOutput":
                shape = tuple(alloc.tensor_shape)
                dtype = mybir.dt.np(alloc.dtype)
                out_names.append(name)
                out_avals.append(jax.core.ShapedArray(shape, dtype))
                zero_outs.append(np.zeros(shape, dtype))
        n_params = len(in_names)
        n_outs = len(out_avals)
        all_in_names = list(in_names) + list(out_names)
        if partition_name is not None:
            all_in_names.append(partition_name)

        def _body(*args):
            operands = list(args)
            if partition_name is not None:
                operands.append(bass2jax.partition_id_tensor())
            outs = bass2jax._bass_exec_p.bind(
                *operands,
                out_avals=tuple(out_avals),
                in_names=tuple(all_in_names),
                out_names=tuple(out_names),
                lowering_input_output_aliases=(),
                sim_require_finite=True,
                sim_require_nnan=True,
                nc=nc,
            )
            return tuple(outs)

        devices = jax.devices()[:n_cores]
        assert len(devices) == n_cores
        mesh = Mesh(np.asarray(devices), ("core",))
        self._mesh = mesh
        donate = tuple(range(n_params, n_params + n_outs))
        self._sharded = jax.jit(
            shard_map(
                _body,
                mesh=mesh,
                in_specs=(PartitionSpec("core"),) * (n_params + n_outs),
                out_specs=(PartitionSpec("core"),) * n_outs,
                check_rep=False,
            ),
            donate_argnums=donate,
            keep_unused=True,
        )
        self.in_names = in_names
        self.out_names = out_names
        self.out_avals = out_avals
        self.zero_outs = zero_outs
        self.n_cores = n_cores

    def _concat_inputs(self, in_maps):
        per_core = [[np.asarray(m[n]) for n in self.in_names] for m in in_maps]
        return [
            np.concatenate([per_core[c][i] for c in range(self.n_cores)], axis=0)
            for i in range(len(self.in_names))
        ]

    def run(self, in_maps, repeats: int = 1):
        """Returns (results_per_core, wall_times_s list of len repeats).

        Inputs and donated zero-output buffers are device_put outside the
        timed region, so wall time ~= dispatch + NEFF execution.
        """
        from jax.sharding import NamedSharding

        mesh = self._mesh
        shard = NamedSharding(mesh, PartitionSpec("core"))
        concat_in = [
            jax.device_put(a, shard) for a in self._concat_inputs(in_maps)
        ]
        rep_zeros = [
            [
                jax.device_put(
                    np.zeros((self.n_cores * z.shape[0], *z.shape[1:]), z.dtype),
                    shard,
                )
                for z in self.zero_outs
            ]
            for _ in range(repeats)
        ]
        jax.block_until_ready(concat_in)
        jax.block_until_ready(rep_zeros)
        times = []
        out_arrs = None
        for r in range(repeats):
            t0 = time.perf_counter()
            out_arrs = self._sharded(*concat_in, *rep_zeros[r])
            jax.block_until_ready(out_arrs)
            times.append(time.perf_counter() - t0)
        results = [
            {
                name: np.asarray(out_arrs[i]).reshape(
                    self.n_cores, *self.out_avals[i].shape
                )[c]
                for i, name in enumerate(self.out_names)
            }
            for c in range(self.n_cores)
        ]
        return results, times


_COMPILED = None


def _get_compiled():
    global _COMPILED
    if _COMPILED is None:
        nc = build_mha()
        _COMPILED = CompiledSpmd(nc, 8)
    return _COMPILED


def kernel(**inputs) -> np.ndarray:
    comp = _get_compiled()
    in_maps = make_host_inputs(**inputs)
    results, _ = comp.run(in_maps, repeats=1)
    return assemble_output(results)
